# revision 17
# baseline (speedup 1.0000x reference)
"""SAN aggregation kernel for Trainium2 (Bass/Tile), 8-core data-parallel.

Problem: out[n,c,h,w] = sum_k w[n, c//8, k, h*W+w] * xpad[n, c, h+dh(k), w+dw(k)]
  x: [8, 64, 128, 128] f32, w: [8, 8, 9, 16384] f32, 3x3 window, pad 1.

Sharding: batch dim N=8 across 8 NeuronCores (1 image per core).

v3 design:
  - The host pre-packs both inputs into the exact fp16 SBUF layout
    (incl. zero halo rows/cols), so every DMA is a plain contiguous
    partition-strided copy and DRAM traffic is halved vs f32.
  - DVE computes ONLY the 9 per-tap products (tensor_mul in the fp16
    2x perf mode); tap SUMMING runs on the otherwise-idle PE: an
    identity [128,128] stationary matmul accumulates the 9 product
    tensors into PSUM f32 (start=k==0 / stop=k==8 per 512-col chunk).
  - ACT (also idle) evicts PSUM f32 -> SBUF fp16 per half-group; the
    stores ride the gpsimd SWDGE queue after all loads; host unpacks.
  - Ramp: the gpsimd SWDGE queue only starts descriptor generation at
    ~7.8us (framework preamble) and serializes ~0.67us per DMA, so the
    first working set (ident, w tap 0, x gl 0 / gl 1) rides the Sync
    and Scalar engines' hardware-DGE queues instead, which are ready
    right after their (shorter) preambles.
  This cuts DVE busy from ~17 passes (~82us) to ~9 passes (~45us),
  with PE/ACT/DMA all hidden behind it.
"""

import sys
import os

for _p in ("/opt/trn_rl_repo", "/root/.axon_site/_ro/trn_rl_repo"):
    if _p not in sys.path and os.path.isdir(_p):
        sys.path.append(_p)

import numpy as np

import concourse.bass as bass
import concourse.bacc as bacc
import concourse.mybir as mybir
import bass_rust
from concourse.tile import TileContext
from concourse.tile_rust import add_dep_helper

F32 = mybir.dt.float32
F16 = mybir.dt.float16

C, H, W = 64, 128, 128
S = H * W          # 16384
CW, GL = 8, 8      # weight channels, share planes
HB = 16            # row blocks
RB = H // HB       # rows per block = 8
XR = RB + 2        # 10 rows incl halo
XW = W + 2         # 130 cols incl left/right zero pad
XGL = XR * XW      # 1300 elements per gl block in x16
SB = RB * W        # 1024 output elems per partition per gl
NG = 2             # gls per compute group
NGRP = GL // NG    # 4 groups
CH = 512           # matmul moving-dim chunk (hw max)

# Interleaved per-partition layout of the combined w+x SBUF image:
#   [w0 w1 w2 | x0 x1 | w3 w4 w5 | x2 x3 | w6 w7 w8 | x4 x5 | x6 x7]
# - each dh-row's 3 w taps are contiguous (stride SB) so the fused
#   3-tap muls can walk them with one AP dim;
# - each x gl-pair is adjacent (stride XGL) for the 4D mul APs;
# - the first DMA delivers w taps 0-2 AND x gl 0-1 as ONE slice.
W_OFF = [0, 1024, 2048, 5672, 6696, 7720, 11344, 12368, 13392]
X_OFF = [3072, 4372, 8744, 10044, 14416, 15716, 17016, 18316]
WX = 9 * SB + GL * XGL  # 19616
# load slices (contiguous [start, end) ranges), in SWDGE issue order:
# w tap 0 + x gl 0-1 (first mul), then w taps 1-8 per-plane racing
# group 0's ~1.14us/tap consumption, then the x pairs.
LOADS = [(0, 1024), (3072, 5672), (1024, 2048), (2048, 3072),
         (5672, 6696), (6696, 7720), (7720, 8744), (11344, 12368),
         (12368, 13392), (13392, 14416), (8744, 11344),
         (14416, 17016), (17016, 19616)]


def _ap(base, dims, extra_offset=0):
    """Copy AP `base`, replace its [step,count] dims, bump offset.

    dims[0] is the partition dim: step "P" substitutes the base AP's own
    partition stride (flat element space, = free width).
    """
    c = base.copy()
    pstep = base.ap[0][0]
    dims = [[pstep if s == "P" else s, n] for s, n in dims]
    c.ap = bass_rust.VecI64Pair(dims)
    if extra_offset:
        c.offset = c.offset + extra_offset
    return c


def build_program():
    nc = bacc.Bacc("TRN2", target_bir_lowering=False, debug=False)
    wx_d = nc.dram_tensor("wx", [128, WX], F16, kind="ExternalInput")
    o_d = nc.dram_tensor("out", [128, GL * SB], F16, kind="ExternalOutput")
    id_d = nc.inline_tensor(np.eye(128, dtype=np.float16), name="ident")

    with TileContext(nc) as tc:
        with tc.tile_pool(name="main", bufs=1) as pool, \
             tc.tile_pool(name="tmps", bufs=4) as tpool, \
             tc.tile_pool(name="evs", bufs=4) as epool, \
             tc.tile_pool(name="ps", bufs=4, space="PSUM") as ppool:
            wx16 = pool.tile([128, WX], F16)
            ident = pool.tile([128, 128], F16)

            # Per-engine program-order pins: the static scheduler
            # reorders same-engine instructions by its own cost model;
            # chain them so issue order == consumption order.
            _prev = {}

            def _pin(eng, d):
                if eng in _prev:
                    add_dep_helper(d.ins, _prev[eng].ins, sync=False,
                                   reason="issue order")
                _prev[eng] = d
                return d

            ENG = {"pool": nc.gpsimd, "sync": nc.sync, "act": nc.scalar}

            def load_ident(q):
                _pin(q, ENG[q].dma_start(
                    out=_ap(ident[:], [["P", 128], [1, 128]]),
                    in_=_ap(id_d.ap(), [[128, 128], [1, 128]])))

            # ALL bulk loads ride the single gpsimd SWDGE queue (it
            # alone sustains ~300 GB/s; concurrent HWDGE queues were
            # measured to steal DRAM-channel bandwidth from it and made
            # the ramp WORSE).  Order per LOADS: the merged first
            # working set (w0 + x gl 0-1 in ONE descriptor), then w
            # taps 1-8 racing just ahead of group 0's ~1.14us/tap
            # consumption, then the x pairs (needed ~10us/group later).
            # Only the tiny identity rides the sync HWDGE queue.
            load_ident("sync")
            for a, b in LOADS:
                _pin("pool", nc.gpsimd.dma_start(
                    out=_ap(wx16[:], [["P", 128], [1, b - a]],
                            extra_offset=a),
                    in_=_ap(wx_d.ap(), [[WX, 128], [1, b - a]],
                            extra_offset=a)))

            def out_dma(gl, src, q):
                """Store one gl, fp16 SBUF -> fp16 DRAM, pushed onto
                the evicting engine's own HWDGE queue: the gpsimd SWDGE
                queue then closes right after the loads, so its ~3us
                ring-teardown drain hides under compute instead of
                sitting at the kernel tail."""
                eng = {"act": nc.scalar, "sync": nc.sync}[q]
                return _pin(q, eng.dma_start(
                    out=_ap(o_d.ap(), [[GL * SB, 128], [1, SB]],
                            extra_offset=gl * SB),
                    in_=_ap(src[:], [["P", 128], [1, SB]])))

            # tap (dh, dw): prod[h', w] = w_k[h', w] * x[r=h'+dh, c'=w+dw]
            # (the x col pads make the dw=0 / dw=2 borders exact zeros).
            for g in range(NGRP):
                g0 = g * NG
                # one PSUM tile per gl (2 banks each) so the two final
                # evictions read different tiles and run on different
                # engines in parallel (the tile framework serializes
                # same-tile readers).
                pss = [ppool.tile([128, SB], F32, tag="ps", name="ps")
                       for _ in range(NG)]
                if g == 0 or g == NGRP - 1:
                    # group 0 races the per-plane w deliveries with
                    # per-tap muls for row 0, then fused rows once
                    # their w planes are resident; the last group
                    # mirrors it with the fused rows first so the
                    # final taps split per gl for an early tail.
                    taps = ((0, 1, 2, "f1", "f2") if g == 0 else
                            ("f0", "f1", 6, 7, 8))
                    for tap in taps:
                        if isinstance(tap, str):
                            dh = int(tap[1])
                            for h in range(NG):
                                t = tpool.tile([128, 3 * SB], F16,
                                               tag="t3", name="t")
                                xv = _ap(wx16[:],
                                         [["P", 128], [1, 3],
                                          [XW, RB], [1, W]],
                                         extra_offset=X_OFF[g0 + h]
                                         + dh * XW)
                                wv = _ap(wx16[:],
                                         [["P", 128], [SB, 3],
                                          [W, RB], [1, W]],
                                         extra_offset=W_OFF[3 * dh])
                                tv = _ap(t[:], [["P", 128], [SB, 3],
                                                [W, RB], [1, W]])
                                _pin("dve", nc.vector.tensor_mul(
                                    out=tv, in0=xv, in1=wv))
                                for tt in range(3):
                                    for c in range(SB // CH):
                                        _pin("pe", nc.tensor.matmul(
                                            out=_ap(pss[h][:],
                                                    [["P", 128],
                                                     [1, CH]],
                                                    extra_offset=c
                                                    * CH),
                                            lhsT=ident[:],
                                            rhs=_ap(t[:],
                                                    [["P", 128],
                                                     [1, CH]],
                                                    extra_offset=tt
                                                    * SB + c * CH),
                                            start=(dh == 0
                                                   and tt == 0),
                                            stop=(dh == 2
                                                  and tt == 2)))
                            continue
                        k = tap
                        dh, dw = divmod(k, 3)
                        # last tap of the last group: split per gl so
                        # gl 6's stop-matmuls and eviction start one
                        # mul earlier.
                        split = (g == NGRP - 1 and k == 8)
                        nparts = NG if split else 1
                        for p in range(nparts):
                            nng = 1 if split else NG
                            t = tpool.tile([128, nng * SB], F16,
                                           tag=f"t{nng}", name="t")
                            xv = _ap(wx16[:], [["P", 128], [XGL, nng],
                                               [XW, RB], [1, W]],
                                     extra_offset=X_OFF[g0 + p]
                                     + dh * XW + dw)
                            wv = _ap(wx16[:], [["P", 128], [0, nng],
                                               [W, RB], [1, W]],
                                     extra_offset=W_OFF[k])
                            tv = _ap(t[:], [["P", 128], [SB, nng],
                                            [W, RB], [1, W]])
                            _pin("dve", nc.vector.tensor_mul(
                                out=tv, in0=xv, in1=wv))
                            for c in range(nng * SB // CH):
                                cc = p * (SB // CH) + c
                                _pin("pe", nc.tensor.matmul(
                                    out=_ap(pss[cc // (SB // CH)][:],
                                            [["P", 128], [1, CH]],
                                            extra_offset=(cc %
                                                          (SB // CH))
                                            * CH),
                                    lhsT=ident[:],
                                    rhs=_ap(t[:], [["P", 128], [1, CH]],
                                            extra_offset=c * CH),
                                    start=(k == 0), stop=(k == 8)))
                else:
                    # later groups have all of w resident: fuse the 3
                    # same-row taps (dw 0..2, x offsets differing by 1)
                    # into ONE per-gl mul — 3 ops of 3072 elems per gl
                    # instead of 4.5 of 2048, saving op overhead.
                    for h in range(NG):
                        for dh in range(3):
                            t = tpool.tile([128, 3 * SB], F16, tag="t3",
                                           name="t")
                            xv = _ap(wx16[:], [["P", 128], [1, 3],
                                               [XW, RB], [1, W]],
                                     extra_offset=X_OFF[g0 + h]
                                     + dh * XW)
                            wv = _ap(wx16[:], [["P", 128], [SB, 3],
                                               [W, RB], [1, W]],
                                     extra_offset=W_OFF[3 * dh])
                            tv = _ap(t[:], [["P", 128], [SB, 3],
                                            [W, RB], [1, W]])
                            _pin("dve", nc.vector.tensor_mul(
                                out=tv, in0=xv, in1=wv))
                            for tap in range(3):
                                for c in range(SB // CH):
                                    _pin("pe", nc.tensor.matmul(
                                        out=_ap(pss[h][:],
                                                [["P", 128], [1, CH]],
                                                extra_offset=c * CH),
                                        lhsT=ident[:],
                                        rhs=_ap(t[:],
                                                [["P", 128], [1, CH]],
                                                extra_offset=tap * SB
                                                + c * CH),
                                        start=(dh == 0 and tap == 0),
                                        stop=(dh == 2 and tap == 2)))
                # evict PSUM -> fp16 SBUF on ACT (DMA cannot read
                # PSUM).  Last group: gl 6 on the by-then-idle DVE in
                # parallel with ACT's gl 7, per 512-chunk so the store
                # wires start as early as possible.
                for h in range(NG):
                    ev = epool.tile([128, SB], F16, tag="ev", name="ev")
                    pv = _ap(pss[h][:], [["P", 128], [1, SB]])
                    if g == NGRP - 1 and h == 0:
                        _pin("dve", nc.vector.tensor_copy(out=ev[:],
                                                          in_=pv))
                        out_dma(g0 + h, ev, "sync")
                    else:
                        _pin("act", nc.scalar.copy(out=ev[:], in_=pv))
                        out_dma(g0 + h, ev, "act")

    nc.compile()
    return nc


_NC_CACHE = None


def _get_nc():
    global _NC_CACHE
    if _NC_CACHE is None:
        _NC_CACHE = build_program()
    return _NC_CACHE


def pack_inputs(x, w):
    """x: [N,64,128,128] f32, w: [N,8,9,16384] f32 ->
    wx: [N,128,WX] f16 (per-core interleaved SBUF image)."""
    N = x.shape[0]
    xq = np.zeros((N, C, H + 2, W + 2), np.float16)
    xq[:, :, 1:H + 1, 1:W + 1] = x
    # [N, hb, cw, gl, r, col]
    xp = np.empty((N, HB, CW, GL, XR, XW), np.float16)
    xv = xq.reshape(N, CW, GL, H + 2, XW)
    for hb in range(HB):
        xp[:, hb] = xv[:, :, :, hb * RB:hb * RB + XR, :]
    xp = xp.reshape(N, 128, GL, XGL)
    wp = np.asarray(w, np.float16).reshape(N, CW, 9, HB, SB).transpose(
        0, 3, 1, 2, 4).reshape(N, 128, 9, SB)  # [N, p, k, sb]
    wx = np.empty((N, 128, WX), np.float16)
    for k in range(9):
        wx[:, :, W_OFF[k]:W_OFF[k] + SB] = wp[:, :, k]
    for gl in range(GL):
        wx[:, :, X_OFF[gl]:X_OFF[gl] + XGL] = xp[:, :, gl]
    return np.ascontiguousarray(wx)


def unpack_output(o):
    """o: [N,128,8192] f16 -> [N,64,128,128] f32."""
    N = o.shape[0]
    v = o.reshape(N, HB, CW, GL, RB, W).transpose(0, 2, 3, 1, 4, 5)
    return np.ascontiguousarray(v.reshape(N, C, H, W)).astype(np.float32)


def kernel(input, weight):
    """input: [8,64,128,128] f32, weight: [8,8,9,16384] f32 ->
    [8,64,128,128] f32."""
    from concourse.bass_utils import run_bass_kernel_spmd

    x = np.asarray(input, dtype=np.float32)
    w = np.asarray(weight, dtype=np.float32)
    N = x.shape[0]
    wx = pack_inputs(x, w)
    nc = _get_nc()
    in_maps = [{"wx": wx[i]} for i in range(N)]
    res = run_bass_kernel_spmd(nc, in_maps, core_ids=list(range(N)))
    o = np.stack([res.results[i]["out"] for i in range(N)])
    return unpack_output(o)


# revision 18
# speedup vs baseline: 1.0006x; 1.0006x over previous
"""SAN aggregation kernel for Trainium2 (Bass/Tile), 8-core data-parallel.

Problem: out[n,c,h,w] = sum_k w[n, c//8, k, h*W+w] * xpad[n, c, h+dh(k), w+dw(k)]
  x: [8, 64, 128, 128] f32, w: [8, 8, 9, 16384] f32, 3x3 window, pad 1.

Sharding: batch dim N=8 across 8 NeuronCores (1 image per core).

v3 design:
  - The host pre-packs both inputs into the exact fp16 SBUF layout
    (incl. zero halo rows/cols), so every DMA is a plain contiguous
    partition-strided copy and DRAM traffic is halved vs f32.
  - DVE computes ONLY the 9 per-tap products (tensor_mul in the fp16
    2x perf mode); tap SUMMING runs on the otherwise-idle PE: an
    identity [128,128] stationary matmul accumulates the 9 product
    tensors into PSUM f32 (start=k==0 / stop=k==8 per 512-col chunk).
  - ACT (also idle) evicts PSUM f32 -> SBUF fp16 per half-group; the
    stores ride the gpsimd SWDGE queue after all loads; host unpacks.
  - Ramp: the gpsimd SWDGE queue only starts descriptor generation at
    ~7.8us (framework preamble) and serializes ~0.67us per DMA, so the
    first working set (ident, w tap 0, x gl 0 / gl 1) rides the Sync
    and Scalar engines' hardware-DGE queues instead, which are ready
    right after their (shorter) preambles.
  This cuts DVE busy from ~17 passes (~82us) to ~9 passes (~45us),
  with PE/ACT/DMA all hidden behind it.
"""

import sys
import os

for _p in ("/opt/trn_rl_repo", "/root/.axon_site/_ro/trn_rl_repo"):
    if _p not in sys.path and os.path.isdir(_p):
        sys.path.append(_p)

import numpy as np

import concourse.bass as bass
import concourse.bacc as bacc
import concourse.mybir as mybir
import bass_rust
from concourse.tile import TileContext
from concourse.tile_rust import add_dep_helper

F32 = mybir.dt.float32
F16 = mybir.dt.float16

C, H, W = 64, 128, 128
S = H * W          # 16384
CW, GL = 8, 8      # weight channels, share planes
HB = 16            # row blocks
RB = H // HB       # rows per block = 8
XR = RB + 2        # 10 rows incl halo
XW = W + 2         # 130 cols incl left/right zero pad
XGL = XR * XW      # 1300 elements per gl block in x16
SB = RB * W        # 1024 output elems per partition per gl
NG = 2             # gls per compute group
NGRP = GL // NG    # 4 groups
CH = 512           # matmul moving-dim chunk (hw max)

# Interleaved per-partition layout of the combined w+x SBUF image:
#   [w0 w1 w2 | x0 x1 | w3 w4 w5 | x2 x3 | w6 w7 w8 | x4 x5 | x6 x7]
# - each dh-row's 3 w taps are contiguous (stride SB) so the fused
#   3-tap muls can walk them with one AP dim;
# - each x gl-pair is adjacent (stride XGL) for the 4D mul APs;
# - the first DMA delivers w taps 0-2 AND x gl 0-1 as ONE slice.
W_OFF = [0, 1024, 2048, 5672, 6696, 7720, 11344, 12368, 13392]
X_OFF = [3072, 4372, 8744, 10044, 14416, 15716, 17016, 18316]
WX = 9 * SB + GL * XGL  # 19616
# load slices (contiguous [start, end) ranges), in SWDGE issue order:
# w tap 0 + x gl 0-1 (first mul), then w taps 1-8 per-plane racing
# group 0's ~1.14us/tap consumption, then the x pairs.
LOADS = [(0, 1024), (3072, 5672), (1024, 2048), (2048, 3072),
         (5672, 6696), (6696, 7720), (7720, 8744), (11344, 12368),
         (12368, 13392), (13392, 14416), (8744, 11344),
         (14416, 17016), (17016, 19616)]


def _ap(base, dims, extra_offset=0):
    """Copy AP `base`, replace its [step,count] dims, bump offset.

    dims[0] is the partition dim: step "P" substitutes the base AP's own
    partition stride (flat element space, = free width).
    """
    c = base.copy()
    pstep = base.ap[0][0]
    dims = [[pstep if s == "P" else s, n] for s, n in dims]
    c.ap = bass_rust.VecI64Pair(dims)
    if extra_offset:
        c.offset = c.offset + extra_offset
    return c


def build_program():
    nc = bacc.Bacc("TRN2", target_bir_lowering=False, debug=False)
    wx_d = nc.dram_tensor("wx", [128, WX], F16, kind="ExternalInput")
    o_d = nc.dram_tensor("out", [128, GL * SB], F16, kind="ExternalOutput")
    id_d = nc.inline_tensor(np.eye(128, dtype=np.float16), name="ident")

    with TileContext(nc) as tc:
        with tc.tile_pool(name="main", bufs=1) as pool, \
             tc.tile_pool(name="tmps", bufs=4) as tpool, \
             tc.tile_pool(name="evs", bufs=4) as epool, \
             tc.tile_pool(name="ps", bufs=4, space="PSUM") as ppool:
            wx16 = pool.tile([128, WX], F16)
            ident = pool.tile([128, 128], F16)

            # Per-engine program-order pins: the static scheduler
            # reorders same-engine instructions by its own cost model;
            # chain them so issue order == consumption order.
            _prev = {}

            def _pin(eng, d):
                if eng in _prev:
                    add_dep_helper(d.ins, _prev[eng].ins, sync=False,
                                   reason="issue order")
                _prev[eng] = d
                return d

            ENG = {"pool": nc.gpsimd, "sync": nc.sync, "act": nc.scalar}

            def load_ident(q):
                _pin(q, ENG[q].dma_start(
                    out=_ap(ident[:], [["P", 128], [1, 128]]),
                    in_=_ap(id_d.ap(), [[128, 128], [1, 128]])))

            # ALL bulk loads ride the single gpsimd SWDGE queue (it
            # alone sustains ~300 GB/s; concurrent HWDGE queues were
            # measured to steal DRAM-channel bandwidth from it and made
            # the ramp WORSE).  Order per LOADS: the merged first
            # working set (w0 + x gl 0-1 in ONE descriptor), then w
            # taps 1-8 racing just ahead of group 0's ~1.14us/tap
            # consumption, then the x pairs (needed ~10us/group later).
            # Only the tiny identity rides the sync HWDGE queue.
            load_ident("sync")
            for a, b in LOADS:
                _pin("pool", nc.gpsimd.dma_start(
                    out=_ap(wx16[:], [["P", 128], [1, b - a]],
                            extra_offset=a),
                    in_=_ap(wx_d.ap(), [[WX, 128], [1, b - a]],
                            extra_offset=a)))

            def out_dma(gl, src, q):
                """Store one gl, fp16 SBUF -> fp16 DRAM, pushed onto
                the evicting engine's own HWDGE queue: the gpsimd SWDGE
                queue then closes right after the loads, so its ~3us
                ring-teardown drain hides under compute instead of
                sitting at the kernel tail."""
                eng = {"act": nc.scalar, "sync": nc.sync}[q]
                return _pin(q, eng.dma_start(
                    out=_ap(o_d.ap(), [[GL * SB, 128], [1, SB]],
                            extra_offset=gl * SB),
                    in_=_ap(src[:], [["P", 128], [1, SB]])))

            # tap (dh, dw): prod[h', w] = w_k[h', w] * x[r=h'+dh, c'=w+dw]
            # (the x col pads make the dw=0 / dw=2 borders exact zeros).
            for g in range(NGRP):
                g0 = g * NG
                # one PSUM tile per gl (2 banks each) so the two final
                # evictions read different tiles and run on different
                # engines in parallel (the tile framework serializes
                # same-tile readers).
                pss = [ppool.tile([128, SB], F32, tag="ps", name="ps")
                       for _ in range(NG)]
                if g == 0 or g == NGRP - 1:
                    # group 0 races the per-plane w deliveries: one mul
                    # per tap over the gl pair (~1.14us/tap matches the
                    # ~0.9us/plane SWDGE cadence).
                    for k in range(9):
                        dh, dw = divmod(k, 3)
                        # last tap of the last group: split per gl so
                        # gl 6's stop-matmuls and eviction start one
                        # mul earlier.
                        split = (g == NGRP - 1 and k == 8)
                        nparts = NG if split else 1
                        for p in range(nparts):
                            nng = 1 if split else NG
                            t = tpool.tile([128, nng * SB], F16,
                                           tag=f"t{nng}", name="t")
                            xv = _ap(wx16[:], [["P", 128], [XGL, nng],
                                               [XW, RB], [1, W]],
                                     extra_offset=X_OFF[g0 + p]
                                     + dh * XW + dw)
                            wv = _ap(wx16[:], [["P", 128], [0, nng],
                                               [W, RB], [1, W]],
                                     extra_offset=W_OFF[k])
                            tv = _ap(t[:], [["P", 128], [SB, nng],
                                            [W, RB], [1, W]])
                            _pin("dve", nc.vector.tensor_mul(
                                out=tv, in0=xv, in1=wv))
                            for c in range(nng * SB // CH):
                                cc = p * (SB // CH) + c
                                _pin("pe", nc.tensor.matmul(
                                    out=_ap(pss[cc // (SB // CH)][:],
                                            [["P", 128], [1, CH]],
                                            extra_offset=(cc %
                                                          (SB // CH))
                                            * CH),
                                    lhsT=ident[:],
                                    rhs=_ap(t[:], [["P", 128], [1, CH]],
                                            extra_offset=c * CH),
                                    start=(k == 0), stop=(k == 8)))
                else:
                    # later groups have all of w resident: fuse the 3
                    # same-row taps (dw 0..2, x offsets differing by 1)
                    # into ONE per-gl mul — 3 ops of 3072 elems per gl
                    # instead of 4.5 of 2048, saving op overhead.
                    for h in range(NG):
                        for dh in range(3):
                            t = tpool.tile([128, 3 * SB], F16, tag="t3",
                                           name="t")
                            xv = _ap(wx16[:], [["P", 128], [1, 3],
                                               [XW, RB], [1, W]],
                                     extra_offset=X_OFF[g0 + h]
                                     + dh * XW)
                            wv = _ap(wx16[:], [["P", 128], [SB, 3],
                                               [W, RB], [1, W]],
                                     extra_offset=W_OFF[3 * dh])
                            tv = _ap(t[:], [["P", 128], [SB, 3],
                                            [W, RB], [1, W]])
                            _pin("dve", nc.vector.tensor_mul(
                                out=tv, in0=xv, in1=wv))
                            for tap in range(3):
                                for c in range(SB // CH):
                                    _pin("pe", nc.tensor.matmul(
                                        out=_ap(pss[h][:],
                                                [["P", 128], [1, CH]],
                                                extra_offset=c * CH),
                                        lhsT=ident[:],
                                        rhs=_ap(t[:],
                                                [["P", 128], [1, CH]],
                                                extra_offset=tap * SB
                                                + c * CH),
                                        start=(dh == 0 and tap == 0),
                                        stop=(dh == 2 and tap == 2)))
                # evict PSUM -> fp16 SBUF on ACT (DMA cannot read
                # PSUM).  Last group: gl 6 on the by-then-idle DVE in
                # parallel with ACT's gl 7, per 512-chunk so the store
                # wires start as early as possible.
                for h in range(NG):
                    ev = epool.tile([128, SB], F16, tag="ev", name="ev")
                    pv = _ap(pss[h][:], [["P", 128], [1, SB]])
                    if g == NGRP - 1 and h == 0:
                        _pin("dve", nc.vector.tensor_copy(out=ev[:],
                                                          in_=pv))
                        out_dma(g0 + h, ev, "sync")
                    else:
                        _pin("act", nc.scalar.copy(out=ev[:], in_=pv))
                        out_dma(g0 + h, ev, "act")

    nc.compile()
    return nc


_NC_CACHE = None


def _get_nc():
    global _NC_CACHE
    if _NC_CACHE is None:
        _NC_CACHE = build_program()
    return _NC_CACHE


def pack_inputs(x, w):
    """x: [N,64,128,128] f32, w: [N,8,9,16384] f32 ->
    wx: [N,128,WX] f16 (per-core interleaved SBUF image)."""
    N = x.shape[0]
    xq = np.zeros((N, C, H + 2, W + 2), np.float16)
    xq[:, :, 1:H + 1, 1:W + 1] = x
    # [N, hb, cw, gl, r, col]
    xp = np.empty((N, HB, CW, GL, XR, XW), np.float16)
    xv = xq.reshape(N, CW, GL, H + 2, XW)
    for hb in range(HB):
        xp[:, hb] = xv[:, :, :, hb * RB:hb * RB + XR, :]
    xp = xp.reshape(N, 128, GL, XGL)
    wp = np.asarray(w, np.float16).reshape(N, CW, 9, HB, SB).transpose(
        0, 3, 1, 2, 4).reshape(N, 128, 9, SB)  # [N, p, k, sb]
    wx = np.empty((N, 128, WX), np.float16)
    for k in range(9):
        wx[:, :, W_OFF[k]:W_OFF[k] + SB] = wp[:, :, k]
    for gl in range(GL):
        wx[:, :, X_OFF[gl]:X_OFF[gl] + XGL] = xp[:, :, gl]
    return np.ascontiguousarray(wx)


def unpack_output(o):
    """o: [N,128,8192] f16 -> [N,64,128,128] f32."""
    N = o.shape[0]
    v = o.reshape(N, HB, CW, GL, RB, W).transpose(0, 2, 3, 1, 4, 5)
    return np.ascontiguousarray(v.reshape(N, C, H, W)).astype(np.float32)


def kernel(input, weight):
    """input: [8,64,128,128] f32, weight: [8,8,9,16384] f32 ->
    [8,64,128,128] f32."""
    from concourse.bass_utils import run_bass_kernel_spmd

    x = np.asarray(input, dtype=np.float32)
    w = np.asarray(weight, dtype=np.float32)
    N = x.shape[0]
    wx = pack_inputs(x, w)
    nc = _get_nc()
    in_maps = [{"wx": wx[i]} for i in range(N)]
    res = run_bass_kernel_spmd(nc, in_maps, core_ids=list(range(N)))
    o = np.stack([res.results[i]["out"] for i in range(N)])
    return unpack_output(o)
